# revision 1
# baseline (speedup 1.0000x reference)
"""HNLoRALinear Trainium2 kernel.

out[b,s,o] = x[b] @ W^T + bias + SCALE * (x[b] @ A[b]) @ B[b]

8 cores = 4 batches x 2 sequence-halves; per-core [1024 tok, 4096 out]
computed transposed (outs on PSUM partitions). Key structure:
  - bf16 everywhere on device (half the DMA bytes of f32, same 1
    col/cycle PE rate, hidden ~97ns weight loads; rel err 2.5e-3 vs the
    2e-2 gate). PSUM accumulates f32.
  - The DMA engine round-robins among in-flight ring entries, so a
    transfer's completion latency ~ (outstanding bytes)/BW. All loads
    are chunked ~128-256KB in need order; ring-depth backpressure keeps
    completion order tracking dispatch order.
  - x (8MB) + early W strips + outputs ride the SP HWDGE queue (w0/w2/w1
    quarter chunks woven between x groups at consumption cadence); x
    groups 1-2 ride the otherwise-idle Activation queue so they progress
    in parallel with group 0 through the DMA-engine ramp; later W strips
    ride the Activation queue as whole-strip DMAs, WAR-released ~3
    strips ahead of need (fewer DMAs = fewer semaphores for the serial
    epilogue teardown).
  - A dozen dummy matmuls on memset scratch burn the PE's ~3us p-state
    ramp (0.65->2.4GHz) while the first DMAs are in flight.
  - Phase 1 overlaps the x load: the lora pre-pass (low = A^T x) plus
    three o-chains (o=0 tracking delivery, o=1 k-rotated by 12, o=2
    lagging 2 groups as always-ready filler) consume x as it lands.
  - Phase 2 runs one [128,512] PSUM bank per chain: 32 accumulating
    matmuls plus one K=128 closing matmul (rows: 16 loraB*SCALE, 1 bias
    via a ones row in `low`, 111 zero-pad so the PE keeps its 128-row
    config; a 17-row stationary forces a ~100ns array reconfig on both
    sides), then a DVE copy to bf16 and a per-half DMA out.
"""
import numpy as np
import ml_dtypes

import concourse.bass as bass  # noqa: F401  (bass must import before tile)
import concourse.mybir as mybir
import concourse.tile as tile
from concourse import bacc
from concourse.bass_utils import run_bass_kernel_spmd

# Problem shapes (hardcoded per contract).
B, S, D_IN, D_OUT, R = 4, 2048, 4096, 4096, 16
SCALE = 32.0 / 16.0
SH = S // 2            # tokens per core
P = 128
KC = D_IN // P         # 32 contraction chunks
O_CHUNKS = D_OUT // P  # 32 output-feature chunks (PSUM partition dim)
TN = 512               # moving-dim token group width (one PSUM bank)
TG = SH // TN          # 2 token groups
XG = 16                # x k-pair groups (2 k-chunks each)
KG = KC // XG          # k-chunks per x group (2)
RA = 128               # augmented rank padded to full PE rows (lora + bias + zeros)
N_EARLY = 3            # o-chunks interleaved with the x load
O1_ROT = 12            # k-chunk rotation for early tile o=1 (starts at g=6)
O2_LAG = 2             # early tile o=2 consumes groups 2 behind delivery
WQ = 4                 # k-quarter chunks per W strip

_cached_nc = None


def _build():
    bf16 = mybir.dt.bfloat16
    f32r = mybir.dt.float32r
    f32 = mybir.dt.float32
    nc = bacc.Bacc(
        "TRN2", target_bir_lowering=False, debug=False, enable_asserts=False
    )
    xt = nc.dram_tensor("xt", [XG, P, KG * SH], bf16, kind="ExternalInput")
    wt = nc.dram_tensor("wt", [O_CHUNKS, P, KC * P], bf16, kind="ExternalInput")
    apk = nc.dram_tensor("apack", [P, KC * R], bf16, kind="ExternalInput")
    ones_d = nc.dram_tensor("ones", [1, SH], bf16, kind="ExternalInput")
    bga = nc.dram_tensor("baug", [RA, D_OUT], bf16, kind="ExternalInput")
    ot_d = nc.dram_tensor("ot", [D_OUT, SH], bf16, kind="ExternalOutput")

    with tile.TileContext(nc) as tc:
        with (
            tc.tile_pool(name="xp", bufs=1) as xp,
            tc.tile_pool(name="wp", bufs=3) as wp,
            tc.tile_pool(name="cp", bufs=1) as cp,
            tc.tile_pool(name="op", bufs=3) as op,
            tc.tile_pool(name="pp", bufs=6, space="PSUM") as pp,
            tc.tile_pool(name="lp", bufs=2, space="PSUM") as lp,
        ):
            # PE p-state warmup: the tensor engine ramps 0.65->1.2->2.4GHz
            # over ~3us of continuous work; a dozen dummy matmuls on a
            # memset scratch burn the ramp while the first DMAs are still
            # in flight, so real matmuls start at full clock.
            scratch = cp.tile([P, 4 + TN], bf16, name="scratch")
            nc.gpsimd.memset(scratch[:], 0.0)
            warm_ps = pp.tile([P, TN], f32, name="ps")
            for _ in range(12):
                nc.tensor.matmul(
                    warm_ps[0:4, :], scratch[:, 0:4], scratch[:, 4 : 4 + TN],
                    start=True, stop=True,
                )

            at = cp.tile([P, KC * R], bf16, name="at")
            nc.sync.dma_start(out=at[:], in_=apk.ap())
            bt = cp.tile([RA, D_OUT], bf16, name="bt")

            w_strips = {}

            def load_w_strip(o, eng=None):
                # 4 k-quarter chunk DMAs per strip; the t=0 chain can start
                # once the first quarter lands. Strips o>=2 ride the Act
                # queue (WAR-released ~3 strips ahead); w0/w1's chunks are
                # interleaved between x chunks on the SP queue below so the
                # x stream keeps priority during the load window.
                wk = wp.tile([P, KC * P], bf16, name="wk")
                w_strips[o] = wk
                if eng is None:
                    return wk
                # phase-2 strips ride as ONE whole-strip DMA: they are
                # WAR-released ~30us ahead with <=3 in flight, so chunk
                # pacing is unnecessary and fewer DMAs means fewer
                # semaphores (the epilogue serially resets every sem used)
                eng.dma_start(out=wk[:], in_=wt.ap()[o])
                return wk

            for o in range(N_EARLY):
                load_w_strip(o)
            _weave = [0, 2, 0, 2, 0, 2, 0, 2, 1, 1, 1, 1]
            _wq = {0: 0, 1: 0, 2: 0}

            def w_chunk(i):
                # i-th woven early-strip quarter chunk, on the SP queue —
                # w0/w2 alternating (both consumed from g<=2), then w1.
                o = _weave[i]
                q = _wq[o]
                _wq[o] += 1
                c0, c1 = q * (KC // WQ) * P, (q + 1) * (KC // WQ) * P
                nc.sync.dma_start(out=w_strips[o][:, c0:c1], in_=wt.ap()[o][:, c0:c1])

            # x^T resident as XG [128, 2048] tiles. Group 0 lands as four
            # [128, 512-col] quarters so the first pre-pass matmul starts
            # ~11us (the DMA engine itself ramps for the first ~10us);
            # later groups use two partition-half DMAs (~256KB) so the
            # ring never holds much undelivered x. w0/w1 chunks are woven
            # in at the cadence the early tiles consume them.
            xgs = []
            for g in range(XG):
                xg = xp.tile([P, KG * SH], bf16, name=f"xg{g}", tag=f"xg{g}")
                if g == 0:
                    for q in range(4):
                        c0, c1 = q * TN, (q + 1) * TN
                        nc.sync.dma_start(out=xg[:, c0:c1], in_=xt.ap()[g][:, c0:c1])
                    w_chunk(0)
                elif g <= 2:
                    # groups 1-2 ride the otherwise-idle Act queue so they
                    # progress in parallel with group 0 during the DMA ramp
                    nc.scalar.dma_start(out=xg[0:64, :], in_=xt.ap()[g][0:64, :])
                    nc.scalar.dma_start(out=xg[64:P, :], in_=xt.ap()[g][64:P, :])
                    w_chunk(g)
                else:
                    nc.sync.dma_start(out=xg[0:64, :], in_=xt.ap()[g][0:64, :])
                    nc.sync.dma_start(out=xg[64:P, :], in_=xt.ap()[g][64:P, :])
                    if g < 12:
                        w_chunk(g)
                xgs.append(xg)
            nc.sync.dma_start(out=bt[:], in_=bga.ap())

            def xsl(k, t):
                base = (k % KG) * SH + t * TN
                return xgs[k // KG][:, base : base + TN]

            # Augmented low-rank activations: rows 0..15 = (x @ A)^T,
            # row 16 = 1 (bias rider; memset fills all 17 rows, copies
            # below overwrite rows 0..15).
            low = cp.tile([RA, SH], bf16, name="low")
            nc.gpsimd.memset(low[:], 0.0)
            nc.sync.dma_start(out=low[R : R + 1, :], in_=ones_d.ap())
            pls = [lp.tile([R, TN], f32, name="pl") for _ in range(TG)]

            ps_early = {
                o: [pp.tile([P, TN], f32, name="ps") for _ in range(TG)]
                for o in range(N_EARLY)
            }

            # Phase 1: consume x groups as they arrive — pre-pass chunks
            # 2g/2g+1 plus the early tiles' chains (o=1 k-rotated so its W
            # strip isn't needed until a few groups in).
            for g in range(XG):
                for k in range(g * KG, (g + 1) * KG):
                    for t in range(TG):
                        nc.tensor.matmul(
                            pls[t][:],
                            at[:, k * R : (k + 1) * R],
                            xsl(k, t),
                            start=(k == 0),
                            stop=(k == KC - 1),
                        )
                # o=0 tracks delivery (k=2g); o=1 is k-rotated (joins at
                # g=6 reading the fresh chunks); o=2 lags delivery by 2
                # groups so its reads always hit resident x — filler work
                # for the slow early-DMA window.
                for o, ks in (
                    (0, range(g * KG, (g + 1) * KG)),
                    (1, range(g * KG, (g + 1) * KG) if g * KG >= O1_ROT else ()),
                    (2, range((g - O2_LAG) * KG, (g - O2_LAG + 1) * KG)
                        if g >= O2_LAG else ()),
                ):
                    for k in ks:
                        first = k == (O1_ROT if o == 1 else 0)
                        for t in range(TG):
                            nc.tensor.matmul(
                                ps_early[o][t][:],
                                w_strips[o][:, k * P : (k + 1) * P],
                                xsl(k, t),
                                start=first,
                                stop=False,
                            )
            # Remaining chunks: o=1 wraps k=0..rot-1, o=2 finishes its
            # lagged tail k=28..31.
            for o, ks in ((1, range(O1_ROT)), (2, range((XG - O2_LAG) * KG, KC))):
                for k in ks:
                    for t in range(TG):
                        nc.tensor.matmul(
                            ps_early[o][t][:],
                            w_strips[o][:, k * P : (k + 1) * P],
                            xsl(k, t),
                            start=False,
                            stop=False,
                        )

            for t in range(TG):
                nc.vector.tensor_copy(low[0:R, t * TN : (t + 1) * TN], pls[t][:])

            def close_tile(o, ps, t):
                # bias + per-sample lora correction as one K=17 f32r matmul
                # accumulated into the same PSUM group.
                nc.tensor.matmul(
                    ps[:],
                    bt[:, o * P : (o + 1) * P],
                    low[:, t * TN : (t + 1) * TN],
                    start=False,
                    stop=True,
                )

            def emit_out(o, otile, t):
                nc.sync.dma_start(
                    out=ot_d.ap()[o * P : (o + 1) * P, t * TN : (t + 1) * TN],
                    in_=otile[:, t * TN : (t + 1) * TN],
                )

            for o in range(N_EARLY):
                otile = op.tile([P, SH], bf16, name="otile")
                for t in range(TG):
                    close_tile(o, ps_early[o][t], t)
                    nc.vector.tensor_copy(
                        otile[:, t * TN : (t + 1) * TN], ps_early[o][t][:]
                    )
                    emit_out(o, otile, t)

            # Phase 2: same-bank chains (t outer), W strips released 3
            # ahead by the wp pool's WAR dependencies.
            for o in range(N_EARLY, O_CHUNKS):
                wk = load_w_strip(o, eng=nc.scalar)
                otile = op.tile([P, SH], bf16, name="otile")
                for t in range(TG):
                    pso = pp.tile([P, TN], f32, name="ps")
                    for k in range(KC):
                        nc.tensor.matmul(
                            pso[:],
                            wk[:, k * P : (k + 1) * P],
                            xsl(k, t),
                            start=(k == 0),
                            stop=False,
                        )
                    close_tile(o, pso, t)
                    nc.vector.tensor_copy(otile[:, t * TN : (t + 1) * TN], pso[:])
                    emit_out(o, otile, t)
    nc.compile()
    return nc


def _get_nc():
    global _cached_nc
    if _cached_nc is None:
        _cached_nc = _build()
    return _cached_nc


def _in_maps(x, weight, bias, lora_A, lora_B):
    bf16 = ml_dtypes.bfloat16
    # W^T packed as [o_chunk, partition, k*128+c]: element (o*128+c, k*128+p)
    # of W -> wt[o, p, k*128+c]; shared by all cores.
    wt = np.ascontiguousarray(
        weight.T.reshape(KC, P, O_CHUNKS, P).transpose(2, 1, 0, 3).reshape(
            O_CHUNKS, P, KC * P
        )
    ).astype(bf16)
    bias = bias.astype(np.float32, copy=False)
    maps = []
    for c in range(8):
        b, h = divmod(c, 2)
        xtc = np.ascontiguousarray(
            x[b, h * SH : (h + 1) * SH, :].T.reshape(XG, KG, P, SH)
            .transpose(0, 2, 1, 3)
            .reshape(XG, P, KG * SH)
        ).astype(bf16)
        apk = np.ascontiguousarray(
            lora_A[b].reshape(KC, P, R).transpose(1, 0, 2).reshape(P, KC * R)
        ).astype(bf16)
        baug = np.concatenate(
            [
                lora_B[b].astype(np.float32) * np.float32(SCALE),
                bias[None, :],
                np.zeros((RA - R - 1, D_OUT), np.float32),
            ],
            axis=0,
        ).astype(bf16)
        maps.append({
            "xt": xtc, "wt": wt, "apack": apk, "baug": baug,
            "ones": np.ones((1, SH), np.float32).astype(bf16),
        })
    return maps


def kernel(x, weight, bias, lora_A, lora_B, _trace=False, _tmpdir=None):
    x = np.asarray(x, dtype=np.float32)
    weight = np.asarray(weight, dtype=np.float32)
    bias = np.asarray(bias, dtype=np.float32)
    lora_A = np.asarray(lora_A, dtype=np.float32)
    lora_B = np.asarray(lora_B, dtype=np.float32)

    nc = _get_nc()
    maps = _in_maps(x, weight, bias, lora_A, lora_B)
    res = run_bass_kernel_spmd(
        nc, maps, list(range(8)), trace=_trace, tmpdir=_tmpdir
    )
    out = np.empty((B, S, D_OUT), np.float32)
    for c in range(8):
        b, h = divmod(c, 2)
        out[b, h * SH : (h + 1) * SH, :] = res.results[c]["ot"].T.astype(np.float32)
    if _trace:
        return out, res
    return out



# revision 2
# speedup vs baseline: 1.0004x; 1.0004x over previous
"""HNLoRALinear Trainium2 kernel — mixed bf16 + fp8(e4m3) DoubleRow.

out[b,s,o] = x[b] @ W^T + bias + SCALE * (x[b] @ A[b]) @ B[b]

8 cores = 4 batches x 2 sequence-halves; per-core [1024 tok, 4096 out]
computed transposed (outs on PSUM partitions). Key structure:
  - Every 512-col matmul instruction costs ~216ns (519 cyc @2.4GHz);
    an fp8(e4m3) DoubleRow instruction does TWO K=128 contractions in
    that time (lhsT [128,2,128], rhs [128,2,512], result =
    A^T@X0 + B^T@X1) = 2x the bf16 FLOP rate.
  - Full-fp8 misses the 2e-2 error gate (2.7e-2 measured), so K is
    SPLIT: N8=17 k-chunks ride plain fp8 (x*16, W*1024 in e4m3), the
    other 15 ride bf16 PRE-SCALED by the same 16/1024 so every PSUM
    product carries scale 2^14. Measured rel err 1.79e-2
    (deterministic inputs; err scales as 2.65%*sqrt(N8/32)*sqrt(2)/1.414).
    Per (o,t) chain: 15 bf16 + 9 DR = 24 instrs vs all-bf16's 33.
  - The lora close rides the LAST DR pair's free slot: stationary
    slot B = 1024*SCALE*B[b] (rows 0..15, packed as w8 strip slot 17),
    moving slot B = low8 (16*low in e4m3, DVE-written from the
    pre-pass PSUM). No separate close matmul, no PE mode/row
    reconfigs anywhere. bias adds on the DVE copy (tensor_scalar:
    psum * 2^-14 + bias[P,1], per-partition AP) -> bf16 out.
  - Pre-pass low' = 2^14*(x @ A) runs as 16 uniform fp8 DR pairs
    (A*1024 in e4m3), interleaved two-per-fp8-unit into the phase-1
    loop; its x copies for the bf16-range chunks ride the Act queue.
  - DMA: x units + early-W quarters woven on the SP queue at the
    cadence 3 tracking chains (lag 0/1/2) consume them; phase-2
    whole strips on the Act queue, WAR-released by the 3-buf pools.
    A dozen [4x512] dummy matmuls burn the PE p-state ramp
    (long-moving shape matters: short warmups let the clock drop
    before real work).
"""
import numpy as np
import ml_dtypes

import concourse.bass as bass  # noqa: F401  (bass must import before tile)
import concourse.mybir as mybir
import concourse.tile as tile
from concourse import bacc
from concourse.bass_utils import run_bass_kernel_spmd

# Problem shapes (hardcoded per contract).
B, S, D_IN, D_OUT, R = 4, 2048, 4096, 4096, 16
SCALE = 32.0 / 16.0
SH = S // 2            # tokens per core
P = 128
KC = D_IN // P         # 32 contraction chunks
O_CHUNKS = D_OUT // P  # 32 output-feature chunks (PSUM partition dim)
TN = 512               # moving-dim token group width (one PSUM bank)
TG = SH // TN          # 2 token groups

N8 = 17                # fp8 k-chunks (odd; k-chunks 0..N8-1)
NB = KC - N8           # bf16 k-chunks (N8..31)
DRP = (N8 + 1) // 2    # DR instrs per (o,t): 5 x-pairs + (chunk N8-1, lora)
NU8 = DRP - 1          # full fp8 pair units
NBU = (NB + 1) // 2    # bf16 delivery units (pairs, last may be single)
SX = 16.0
SW = 1024.0
SPROD = SX * SW        # 2^14 product scale
N_EARLY = 3            # tracking chains with lag 0/1/2

DR = mybir.MatmulPerfMode.DoubleRow

_cached_nc = None


def _bf_unit_chunks(j):
    """bf16 unit j -> list of local chunk idx (0-based within bf16 block)."""
    c0 = 2 * j
    return [c0] if c0 + 1 >= NB else [c0, c0 + 1]


def _build():
    fp8 = mybir.dt.float8e4
    bf16 = mybir.dt.bfloat16
    f32 = mybir.dt.float32
    nc = bacc.Bacc(
        "TRN2", target_bir_lowering=False, debug=False, enable_asserts=False
    )
    xbd = nc.dram_tensor("xb", [P, NB, SH], bf16, kind="ExternalInput")
    x8d = nc.dram_tensor("x8", [KC // 2, P, 2, SH], fp8, kind="ExternalInput")
    wbd = nc.dram_tensor("wb", [O_CHUNKS, P, NB, P], bf16, kind="ExternalInput")
    w8d = nc.dram_tensor("w8", [O_CHUNKS, P, N8 + 1, P], fp8, kind="ExternalInput")
    a8d = nc.dram_tensor("a8", [P, KC, R], fp8, kind="ExternalInput")
    biasd = nc.dram_tensor("biasp", [P, O_CHUNKS], f32, kind="ExternalInput")
    ot_d = nc.dram_tensor("ot", [D_OUT, SH], bf16, kind="ExternalOutput")

    with tile.TileContext(nc) as tc:
        with (
            tc.tile_pool(name="xp", bufs=1) as xp,
            tc.tile_pool(name="wbp", bufs=3) as wbp,
            tc.tile_pool(name="w8p", bufs=3) as w8p,
            tc.tile_pool(name="cp", bufs=1) as cp,
            tc.tile_pool(name="op", bufs=3) as op,
            tc.tile_pool(name="pp", bufs=6, space="PSUM") as pp,
            tc.tile_pool(name="lp", bufs=2, space="PSUM") as lp,
        ):
            # PE p-state warmup while the first DMAs are in flight.
            scratch = cp.tile([P, 4 + TN], bf16, name="scratch")
            nc.gpsimd.memset(scratch[:], 0.0)
            warm_ps = pp.tile([P, TN], f32, name="ps")
            for _ in range(12):
                nc.tensor.matmul(
                    warm_ps[0:4, :], scratch[:, 0:4], scratch[:, 4 : 4 + TN],
                    start=True, stop=True,
                )

            a8t = cp.tile([P, KC, R], fp8, name="a8t")
            nc.scalar.dma_start(out=a8t[:], in_=a8d.ap())
            # fp8 x pairs used only by the pre-pass (pairs NU8..KC/2-1):
            # no delivery deadline, ride the Act queue during phase 1.
            x8e = []
            for i in range(NU8, KC // 2):
                xt = cp.tile([P, 2, SH], fp8, name=f"x8e{i}")
                nc.scalar.dma_start(out=xt[:], in_=x8d.ap()[i])
                x8e.append(xt)
            bias_t = cp.tile([P, O_CHUNKS], f32, name="bias_t")
            nc.scalar.dma_start(out=bias_t[:], in_=biasd.ap())

            # xlow: slot 0 = x chunk N8-1 (DMA), slot 1 = low8 (DVE later);
            # memset zeroes the dead partitions of slot 1.
            xlow = cp.tile([P, 2, SH], fp8, name="xlow")
            nc.gpsimd.memset(xlow[:], 0.0)

            wb_strips, w8_strips = {}, {}

            def alloc_strips(o, eng=None):
                wbt = wbp.tile([P, NB, P], bf16, name="wbt")
                w8t = w8p.tile([P, N8 + 1, P], fp8, name="w8t")
                wb_strips[o], w8_strips[o] = wbt, w8t
                if eng is not None:
                    eng.dma_start(out=wbt[:], in_=wbd.ap()[o])
                    eng.dma_start(out=w8t[:], in_=w8d.ap()[o])

            for o in range(N_EARLY):
                alloc_strips(o)

            # Early-strip weave on the SP queue: Wb quarters at the cadence
            # the lagged tracking chains consume them, W8 strips late.
            _wbq = [round(i * NB / 4) for i in range(5)]  # quarter boundaries

            def wv_wb(o, q):
                c0, c1 = _wbq[q], _wbq[q + 1]
                nc.sync.dma_start(
                    out=wb_strips[o][:, c0:c1, :], in_=wbd.ap()[o][:, c0:c1, :]
                )

            def wv_w8(o):
                nc.sync.dma_start(out=w8_strips[o][:], in_=w8d.ap()[o])

            # x units in consumption order: NBU bf16 units then NU8 fp8 pairs.
            xbu, x8u = [], []

            def deliver_unit(g):
                if g < NBU:
                    cs = _bf_unit_chunks(g)
                    xt = xp.tile([P, len(cs), SH], bf16, name=f"xb{g}", tag=f"xb{g}")
                    if g == 0:
                        for c in range(len(cs)):
                            for t in range(TG):
                                nc.sync.dma_start(
                                    out=xt[:, c, t * TN : (t + 1) * TN],
                                    in_=xbd.ap()[:, cs[c], t * TN : (t + 1) * TN],
                                )
                    else:
                        nc.sync.dma_start(
                            out=xt[:], in_=xbd.ap()[:, cs[0] : cs[-1] + 1, :]
                        )
                    xbu.append(xt)
                else:
                    i = g - NBU
                    xt = xp.tile([P, 2, SH], fp8, name=f"x8{i}", tag=f"x8{i}")
                    nc.sync.dma_start(out=xt[:], in_=x8d.ap()[i])
                    x8u.append(xt)

            # Weave emission: chain o (lag o) first consumes Wb quarter q's
            # chunks at unit floor(_wbq[q]/2) + o; emit the DMA one unit
            # ahead of that (clamped into range).
            NUNITS = NBU + NU8
            _weave = {}
            for o in range(N_EARLY):
                for q in range(4):
                    g = _wbq[q] // 2 + o - (1 if q else 0)
                    g = min(max(g, 0), NUNITS - 1)
                    _weave.setdefault(g, []).append((wv_wb, o, q))
                g = min(NBU + o - 1, NUNITS - 1)
                _weave.setdefault(g, []).append((wv_w8, o))

            pls = [lp.tile([R, TN], f32, name="pl") for _ in range(TG)]
            ps_early = {
                o: [pp.tile([P, TN], f32, name="ps") for _ in range(TG)]
                for o in range(N_EARLY)
            }
            started = set()

            def term(o, ps, t, lhsT, rhs, perf_mode=None, stop=False):
                first = (o, t) not in started
                if first:
                    started.add((o, t))
                nc.tensor.matmul(ps[t][:], lhsT, rhs, start=first, stop=stop,
                                 perf_mode=perf_mode)

            def chain_unit(o, ps, g):
                if g < NBU:
                    for c in _bf_unit_chunks(g):
                        for t in range(TG):
                            term(o, ps, t,
                                 wb_strips[o][:, c, :],
                                 xbu[g][:, c - 2 * g, t * TN : (t + 1) * TN])
                else:
                    i = g - NBU
                    for t in range(TG):
                        term(o, ps, t,
                             w8_strips[o][:, 2 * i : 2 * i + 2, :],
                             x8u[i][:, :, t * TN : (t + 1) * TN],
                             perf_mode=DR)

            npairs = KC // 2
            pre_state = {"n": 0}

            def prepass_pair(i):
                xt = x8u[i] if i < NU8 else x8e[i - NU8]
                first = pre_state["n"] == 0
                pre_state["n"] += 1
                last = pre_state["n"] == npairs
                pre_state[i] = True
                for t in range(TG):
                    nc.tensor.matmul(
                        pls[t][:], a8t[:, 2 * i : 2 * i + 2, :],
                        xt[:, :, t * TN : (t + 1) * TN],
                        start=first, stop=last, perf_mode=DR,
                    )

            # Phase 1: deliver x units; 3 tracking chains at lags 0/1/2
            # plus the pre-pass consume them as they land. Pre-pass pairs
            # ride the fp8 units (2 pairs per unit: the SP-delivered one
            # plus one Act-delivered extra).
            for g in range(NUNITS):
                for call in _weave.get(g, ()):
                    call[0](*call[1:])
                deliver_unit(g)
                chain_unit(0, ps_early[0], g)
                if g >= 1:
                    chain_unit(1, ps_early[1], g - 1)
                if g >= 2:
                    chain_unit(2, ps_early[2], g - 2)
                if g >= NBU:
                    prepass_pair(g - NBU)
                    e = NU8 + (g - NBU)
                    if e < npairs:
                        prepass_pair(e)
            nc.sync.dma_start(out=xlow[:, 0, :], in_=x8d.ap()[NU8][:, 0, :])
            chain_unit(1, ps_early[1], NUNITS - 1)
            for g in (NUNITS - 2, NUNITS - 1):
                chain_unit(2, ps_early[2], g)

            # pre-pass leftovers (only when unit counts don't cover all
            # pairs), then low8 = pls * 2^-10 (= 16 * low) into xlow slot 1.
            for i in range(npairs):
                if i not in pre_state:
                    prepass_pair(i)
            for t in range(TG):
                nc.vector.tensor_scalar_mul(
                    xlow[0:R, 1, t * TN : (t + 1) * TN], pls[t][:], 1.0 / SW
                )

            def final_dr(o, ps, t):
                term(o, ps, t,
                     w8_strips[o][:, N8 - 1 : N8 + 1, :],
                     xlow[:, :, t * TN : (t + 1) * TN],
                     perf_mode=DR, stop=True)

            def copy_out(o, otile, ps, t):
                nc.vector.tensor_scalar(
                    otile[:, t * TN : (t + 1) * TN],
                    ps[t][:],
                    1.0 / SPROD,
                    bias_t[:, o : o + 1],
                    mybir.AluOpType.mult,
                    mybir.AluOpType.add,
                )

            def emit_out(o, otile):
                nc.sync.dma_start(
                    out=ot_d.ap()[o * P : (o + 1) * P, :], in_=otile[:]
                )

            for o in range(N_EARLY):
                otile = op.tile([P, SH], bf16, name="otile")
                for t in range(TG):
                    final_dr(o, ps_early[o], t)
                    copy_out(o, otile, ps_early[o], t)
                emit_out(o, otile)

            # Phase 2: whole strips on the Act queue, WAR-released ahead.
            for o in range(N_EARLY, O_CHUNKS):
                alloc_strips(o, eng=nc.scalar)
                otile = op.tile([P, SH], bf16, name="otile")
                pso = [pp.tile([P, TN], f32, name="ps") for _ in range(TG)]
                for t in range(TG):
                    for c in range(NB):
                        nc.tensor.matmul(
                            pso[t][:], wb_strips[o][:, c, :],
                            xbu[c // 2][:, c % 2, t * TN : (t + 1) * TN],
                            start=(c == 0), stop=False,
                        )
                    for i in range(NU8):
                        nc.tensor.matmul(
                            pso[t][:], w8_strips[o][:, 2 * i : 2 * i + 2, :],
                            x8u[i][:, :, t * TN : (t + 1) * TN],
                            start=False, stop=False, perf_mode=DR,
                        )
                    nc.tensor.matmul(
                        pso[t][:], w8_strips[o][:, N8 - 1 : N8 + 1, :],
                        xlow[:, :, t * TN : (t + 1) * TN],
                        start=False, stop=True, perf_mode=DR,
                    )
                    copy_out(o, otile, pso, t)
                emit_out(o, otile)
    nc.compile()
    return nc


def _get_nc():
    global _cached_nc
    if _cached_nc is None:
        _cached_nc = _build()
    return _cached_nc


def _in_maps(x, weight, bias, lora_A, lora_B):
    f8 = ml_dtypes.float8_e4m3
    bf = ml_dtypes.bfloat16

    wT = weight.T.astype(np.float32) * np.float32(SW)  # [D_IN, D_OUT]
    w8c = wT[: N8 * P].astype(f8)
    wbc = wT[N8 * P :].astype(bf)
    # [o, p, j, m]: strip[j] = W^T chunk; fp8 strip slot N8 = 1024*SCALE*B.
    w8base = np.ascontiguousarray(
        w8c.reshape(N8, P, O_CHUNKS, P).transpose(2, 1, 0, 3)
    )
    wb_strip = np.ascontiguousarray(
        wbc.reshape(NB, P, O_CHUNKS, P).transpose(2, 1, 0, 3)
    )
    bias_p = np.ascontiguousarray(bias.astype(np.float32).reshape(O_CHUNKS, P).T)

    maps = []
    w8_by_batch = {}
    for c in range(8):
        b, h = divmod(c, 2)
        if b not in w8_by_batch:
            b8 = np.zeros((O_CHUNKS, P, 1, P), f8)
            bb = (lora_B[b].astype(np.float32) * np.float32(SW * SCALE)).astype(f8)
            # bb [R, D_OUT] -> [o, r, m] rows 0..15
            b8[:, 0:R, 0, :] = bb.reshape(R, O_CHUNKS, P).transpose(1, 0, 2)
            w8_by_batch[b] = np.ascontiguousarray(
                np.concatenate([w8base, b8], axis=2)
            )
        xh = x[b, h * SH : (h + 1) * SH, :].T.astype(np.float32) * np.float32(SX)
        x8 = np.ascontiguousarray(
            xh.astype(f8).reshape(KC // 2, 2, P, SH).transpose(0, 2, 1, 3)
        )
        xb = np.ascontiguousarray(
            xh[N8 * P :].astype(bf).reshape(NB, P, SH).transpose(1, 0, 2)
        )
        a_s = lora_A[b].astype(np.float32) * np.float32(SW)
        a8 = np.ascontiguousarray(
            a_s.astype(f8).reshape(KC, P, R).transpose(1, 0, 2)
        )
        maps.append({
            "xb": xb, "x8": x8,
            "wb": wb_strip, "w8": w8_by_batch[b],
            "a8": a8, "biasp": bias_p,
        })
    return maps


def kernel(x, weight, bias, lora_A, lora_B, _trace=False, _tmpdir=None):
    x = np.asarray(x, dtype=np.float32)
    weight = np.asarray(weight, dtype=np.float32)
    bias = np.asarray(bias, dtype=np.float32)
    lora_A = np.asarray(lora_A, dtype=np.float32)
    lora_B = np.asarray(lora_B, dtype=np.float32)

    nc = _get_nc()
    maps = _in_maps(x, weight, bias, lora_A, lora_B)
    res = run_bass_kernel_spmd(
        nc, maps, list(range(8)), trace=_trace, tmpdir=_tmpdir
    )
    out = np.empty((B, S, D_OUT), np.float32)
    for c in range(8):
        b, h = divmod(c, 2)
        out[b, h * SH : (h + 1) * SH, :] = res.results[c]["ot"].T.astype(np.float32)
    if _trace:
        return out, res
    return out
